# revision 53
# baseline (speedup 1.0000x reference)
"""GQA attention (B=2,S=1024,HID=2048,NH=32,NKV=8,HD=64) on 8 TRN2 cores.

Sharding: core c -> batch b=c//4, head-group g=c%4 (8 q heads / 2 kv heads).
Core computes partial out[b] = attn(heads of g) @ Wo[rows of g]; host sums the
4 row-parallel partials per batch.

Head pairing: local q heads are reordered [0,4,1,5,2,6,3,7] so q chunk mc
holds (kv0 head mc) on partitions 0:64 and (kv1 head mc) on 64:128. K proj
then emits both kv heads in ONE [128,S] chunk (kv0 rows 0:64, kv1 rows
64:128) with no replication, and scores use krep[r:r+64] with r=64*(h%2).

Device dataflow (matmuls bf16 -> fp32 PSUM), software-pipelined so the PE
never sits on an ACT/DVE result (the PE clock only ramps to 2.4GHz under
continuous wait-free execution):
  K proj -> V proj (covers ACT copy of K) -> rot(K) -> V transposes
  -> Q0..Q3 with rot of the previous chunk interleaved
  -> per head h: sc(0); for kc: [sc(kc+1); exp(kc) on ACT; PV(kc)]
     every attention matmul runs the full 128x128 array: scores use
     zero-padded krep variants, PV uses full-128 [V|ones|V] lhsT slices
     with the softmax denominator fused (row 64 even / row 0 odd)
  -> normalize: DVE reciprocal -> gpsimd partition_broadcast -> DVE mult;
     the last pair stages through SBUF so the PSUM pools close early and
     head 7's chain overlaps the out-projection's early accumulations
  -> out proj (stationary Wo) -> DMA out [2048,1024] f32 partials
Weights are host-packed into the SBUF tile layouts so every input DMA is a
contiguous 128-partition copy (fast issue, few descriptors).
"""

import numpy as np
import ml_dtypes

import concourse.bass as bass
import concourse.bacc as bacc
import concourse.mybir as mybir
from concourse.tile import TileContext
from concourse.bass_utils import run_bass_kernel_spmd
from concourse.masks import make_identity

B, S, HID = 2, 1024, 2048
NH, NKV, HD = 32, 8, 64
G = 4                      # head groups (tensor-parallel degree per batch)
QH = NH // G               # 8 q heads per core
KVH = NKV // G             # 2 kv heads per core
QD = QH * HD               # 512
ROPE_BASE = 10000.0
BF16 = mybir.dt.bfloat16
F32 = mybir.dt.float32
NEG_BIG = float(np.finfo(np.float32).min)

KC = S // 128              # 8 k-token chunks
HC = HID // 128            # 16 hidden chunks
QC = QD // 128             # 4 q-dim chunks (head pairs)

LAST_RESULT = None
_CACHE = {}


def _build(use_mask: bool) -> bass.Bass:
    nc = bacc.Bacc(None, target_bir_lowering=False)
    # weights arrive pre-packed on host into the SBUF tile layout so every
    # DMA is a contiguous 128-partition copy (few descriptors, fast issue)
    hsT_d = nc.dram_tensor("hsT", [HID, S], BF16, kind="ExternalInput")
    wq_d = nc.dram_tensor("wq", [128, QC * HC * 128], BF16,
                          kind="ExternalInput")
    wk_d = nc.dram_tensor("wk", [128, HC * KVH * HD], BF16,
                          kind="ExternalInput")
    wv_d = nc.dram_tensor("wv", [128, HC * KVH * HD], BF16,
                          kind="ExternalInput")
    wo_d = nc.dram_tensor("wo", [128, QC * HID], BF16, kind="ExternalInput")
    cos_d = nc.dram_tensor("cos2", [128, S], F32, kind="ExternalInput")
    sin_d = nc.dram_tensor("sin2", [128, S], F32, kind="ExternalInput")
    perm_d = nc.dram_tensor("permT", [128, 128], BF16, kind="ExternalInput")
    if use_mask:
        mask_d = nc.dram_tensor("maskT", [S, S], BF16, kind="ExternalInput")
    out_d = nc.dram_tensor("out", [HID, S], F32, kind="ExternalOutput")

    with TileContext(nc) as tc:
        with (
            tc.tile_pool(name="resid", bufs=1) as rp,
            tc.tile_pool(name="work", bufs=2) as wp,
            tc.tile_pool(name="exps", bufs=4) as ep,
            tc.tile_pool(name="outs", bufs=3) as op_,
        ):
            # ---- input DMAs: small early tensors, then hsT (K/V proj gate on
            # it), then wq, then wo (only needed at the end) ----
            wk = rp.tile([128, HC * KVH * HD], BF16, tag="wk")
            nc.sync.dma_start(out=wk[:], in_=wk_d[:, :])
            hsT = []
            for k in range(HC):
                t = rp.tile([128, S], BF16, tag=f"hsT{k}")
                nc.sync.dma_start(out=t[:], in_=hsT_d[k * 128:(k + 1) * 128, :])
                hsT.append(t)
            permT = rp.tile([128, 128], BF16, tag="permT")
            nc.sync.dma_start(out=permT[:], in_=perm_d[:, :])
            cos2 = rp.tile([128, S], F32, tag="cos2")
            nc.sync.dma_start(out=cos2[:], in_=cos_d[:, :])
            sin2 = rp.tile([128, S], F32, tag="sin2")
            nc.sync.dma_start(out=sin2[:], in_=sin_d[:, :])
            wv = rp.tile([128, HC * KVH * HD], BF16, tag="wv")
            nc.sync.dma_start(out=wv[:], in_=wv_d[:, :])
            # wq packed mc-major: block mc holds all 16 hid-chunks of the
            # 128 q dims of pair mc, so qproj(mc) gates only on its block
            wqb = rp.tile([128, QC * HC * 128], BF16, tag="wqb")
            for mc in range(QC):
                nc.sync.dma_start(
                    out=wqb[:, mc * HC * 128:(mc + 1) * HC * 128],
                    in_=wq_d[:, mc * HC * 128:(mc + 1) * HC * 128],
                )
            wo = rp.tile([128, QC * HID], BF16, tag="wo")
            nc.sync.dma_start(out=wo[:], in_=wo_d[:, :])
            if use_mask:
                maskT = rp.tile([128, KC * S], BF16, tag="maskT")
                nc.sync.dma_start(
                    out=maskT[:].rearrange("p (k q) -> p k q", k=KC),
                    in_=mask_d[:, :].rearrange("(k p) q -> p k q", p=128),
                )
            # ---- persistent intermediates ----
            qrot = rp.tile([128, QC * S], BF16, tag="qrot")
            # krep variants zero-padded to full 128 contraction rows: even
            # heads use [K_kv0; 0], odd use [0; K_kv1] so every scores matmul
            # runs the full 128x128 PE array (zeros kill the other head's q).
            krepE = rp.tile([128, S], BF16, tag="krepE")
            nc.any.memset(krepE[64:128, :], 0.0)
            krepO = rp.tile([128, S], BF16, tag="krepO")
            nc.any.memset(krepO[0:64, :], 0.0)
            # PV lhsT tiles, full 128 cols per kc chunk so every PV matmul
            # runs the whole PE array with the softmax denominator fused:
            #   vaugE = [V_kv0 | ones]: PV rows 0:64, denominator on row 64+
            #   vaugO = [ones | V_kv1]: denominator on rows 0:63, PV 64:128
            vaugE = rp.tile([128, KC * 128], BF16, tag="vaugE")
            nc.any.memset(vaugE[:], 1.0)
            vaugO = rp.tile([128, KC * 128], BF16, tag="vaugO")
            nc.any.memset(vaugO[:], 1.0)
            attnT = rp.tile([128, QC * S], BF16, tag="attnT")
            ident = rp.tile([128, 128], BF16, tag="ident")
            make_identity(nc, ident[:])
            # SBUF staging for the last pair so the attention PSUM pools can
            # close right after the final PV (normalize finishes during the
            # out-projection instead of serializing before it)
            lastU = {6: rp.tile([128, S], F32, tag="lastU6", name="lastU6"),
                     7: rp.tile([128, S], F32, tag="lastU7", name="lastU7")}
            lastD = rp.tile([128, S], F32, tag="lastD")

            # ================= projections + rope =================
            # pj: 3 bufs x [128,S] f32 (12KB/part) + tp: 2 x [128,S] bf16
            # (4KB) = 16KB PSUM. Each tp tile owns a full 2KB zero region so
            # the V transposes don't invalidate each other.
            with (
                tc.tile_pool(name="pj", bufs=3, space="PSUM") as pj,
                tc.tile_pool(name="tp", bufs=2, space="PSUM") as tpp,
            ):
                def proj(w_ap_fn, tag):
                    ps = pj.tile([128, S], F32, tag="pj", name=f"ps_{tag}")
                    for ns in range(2):
                        for k in range(HC):
                            nc.tensor.matmul(
                                ps[:, ns * 512:(ns + 1) * 512],
                                w_ap_fn(k),
                                hsT[k][:, ns * 512: ns * 512 + 512],
                                start=(k == 0), stop=(k == HC - 1),
                            )
                    return ps

                def raw_copy(ps, tag):
                    raw = wp.tile([128, S], BF16, tag="raw", name=f"raw_{tag}")
                    nc.scalar.activation(
                        raw[:], ps[:], mybir.ActivationFunctionType.Copy
                    )
                    return raw

                def rot_combine(raw, dsts, tag):
                    """PE rotate-half matmul, then DVE combine with cos/sin.
                    dsts: list of (dst_ap, row_lo, row_hi) receiving the
                    rope'd rows [row_lo:row_hi]."""
                    ps_rot = pj.tile([128, S], F32, tag="pj", name=f"rot_{tag}")
                    for ns in range(2):
                        nc.tensor.matmul(
                            ps_rot[:, ns * 512:(ns + 1) * 512],
                            permT[:],
                            raw[:, ns * 512:(ns + 1) * 512],
                            start=True, stop=True,
                        )
                    t1 = wp.tile([128, S], F32, tag="t1")
                    nc.vector.tensor_tensor(
                        t1[:], raw[:], cos2[:], mybir.AluOpType.mult
                    )
                    t2 = wp.tile([128, S], F32, tag="t2")
                    nc.vector.tensor_tensor(
                        t2[:], ps_rot[:], sin2[:], mybir.AluOpType.mult
                    )
                    for dst_ap, lo, hi in dsts:
                        nc.vector.tensor_tensor(
                            dst_ap, t1[lo:hi, :], t2[lo:hi, :],
                            mybir.AluOpType.add,
                        )

                # K proj; ACT copy overlaps the V projection
                ps_k = proj(lambda k: wk[:, k * 128:(k + 1) * 128], "k")
                raw_k = raw_copy(ps_k, "k")
                # V proj as V^T (stationary wv), rot K covered by it
                ps_vt = proj(lambda k: wv[:, k * 128:(k + 1) * 128], "v")
                rot_combine(raw_k, [
                    (krepE[0:64, :], 0, 64),
                    (krepO[64:128, :], 64, 128),
                ], "k")
                vt_sb = wp.tile([128, S], BF16, tag="vts")
                nc.scalar.activation(
                    vt_sb[:], ps_vt[:], mybir.ActivationFunctionType.Copy
                )
                # PE-transpose each token chunk into vaugE/vaugO; each
                # transpose gets its own PSUM bank (2-tile rotation)
                for t in range(KC):
                    ps_tr = tpp.tile([128, S], BF16, tag="tp", name=f"tr{t}")
                    nc.tensor.transpose(
                        ps_tr[:, 0:128], vt_sb[:, t * 128:(t + 1) * 128],
                        ident[:]
                    )
                    nc.vector.tensor_copy(
                        vaugE[:, t * 128:t * 128 + 64], ps_tr[:, 0:64]
                    )
                    nc.vector.tensor_copy(
                        vaugO[:, t * 128 + 64:(t + 1) * 128], ps_tr[:, 64:128]
                    )
                def wq_ap(mc):
                    return lambda k: wqb[:, mc * HC * 128 + k * 128:
                                         mc * HC * 128 + (k + 1) * 128]

                ps_q = [None] * QC
                raw_q = [None] * QC
                ps_q[0] = proj(wq_ap(0), "q0")
                raw_q[0] = raw_copy(ps_q[0], "q0")
                # Q1..Q3 with rot of the previous chunk interleaved
                def qdst(mc):
                    return [(qrot[:, mc * S:(mc + 1) * S], 0, 128)]

                for mc in range(1, QC):
                    ps_q[mc] = proj(wq_ap(mc), f"q{mc}")
                    raw_q[mc] = raw_copy(ps_q[mc], f"q{mc}")
                    rot_combine(raw_q[mc - 1], qdst(mc - 1), f"q{mc-1}")
                rot_combine(raw_q[QC - 1], qdst(QC - 1), f"q{QC-1}")

            # ================= attention =================
            # PSUM: st 2 x [128,S] f32 (8KB) + av 2 x [128,S] f32 (8KB) = 16KB
            def normalize(h, dsrc, psrc):
                """Denominator row dr of dsrc -> reciprocal -> broadcast ->
                scale psrc's PV rows into attnT. Processed in two 512-col
                halves so the reciprocal/broadcast/multiply pipeline and the
                PSUM accumulator frees ~2us sooner."""
                mc, par = h // 2, h % 2
                r = par * 64
                dr = 64 if par == 0 else 0
                recip = wp.tile([128, S], F32, tag="recip", name=f"recip{h}")
                recip0 = wp.tile([1, S], F32, tag="recip0",
                                 name=f"recip0_{h}")
                bc = wp.tile([128, S], F32, tag="bcast", name=f"bc{h}")
                for hf in range(2):
                    cs = hf * 512
                    nc.vector.reciprocal(
                        recip[dr:dr + 1, cs:cs + 512],
                        dsrc[dr:dr + 1, cs:cs + 512],
                    )
                    if dr != 0:
                        nc.gpsimd.dma_start(
                            out=recip0[0:1, cs:cs + 512],
                            in_=recip[dr:dr + 1, cs:cs + 512],
                        )
                        rsrc = recip0[0:1, cs:cs + 512]
                    else:
                        rsrc = recip[0:1, cs:cs + 512]
                    nc.gpsimd.partition_broadcast(bc[:, cs:cs + 512], rsrc)
                    nc.vector.tensor_tensor(
                        attnT[r:r + 64, mc * S + cs:mc * S + cs + 512],
                        psrc[r:r + 64, cs:cs + 512], bc[r:r + 64, cs:cs + 512],
                        mybir.AluOpType.mult,
                    )

            with (
                tc.tile_pool(name="st", bufs=2, space="PSUM") as stp,
                tc.tile_pool(name="av", bufs=2, space="PSUM") as avp,
            ):
                for h in range(QH):
                    mc = h // 2                  # q chunk / pair
                    par = h % 2                  # kv head = parity
                    r = par * 64                 # partition row base
                    krep = krepE if par == 0 else krepO

                    def scores(kc):
                        ps_st = stp.tile([128, S], F32, tag="st",
                                         name=f"st_h{h}k{kc}")
                        for ns in range(2):
                            nc.tensor.matmul(
                                ps_st[:, ns * 512:(ns + 1) * 512],
                                krep[:, kc * 128:(kc + 1) * 128],
                                qrot[:,
                                     mc * S + ns * 512: mc * S + ns * 512 + 512],
                                start=True, stop=True,
                            )
                        return ps_st

                    ps_at = avp.tile([128, S], F32, tag="av", name=f"av_h{h}")
                    st_tiles = {0: scores(0)}
                    for kc in range(KC):
                        if kc + 1 < KC:
                            st_tiles[kc + 1] = scores(kc + 1)
                        ps_st = st_tiles.pop(kc)
                        if use_mask:
                            nc.vector.tensor_tensor(
                                ps_st[:], ps_st[:],
                                maskT[:, kc * S:(kc + 1) * S],
                                mybir.AluOpType.add,
                            )
                        ex = ep.tile([128, S], BF16, tag="ex",
                                     name=f"ex_h{h}k{kc}")
                        nc.scalar.activation(
                            ex[:], ps_st[:], mybir.ActivationFunctionType.Exp
                        )
                        va = vaugE if par == 0 else vaugO
                        for ns in range(2):
                            nc.tensor.matmul(
                                ps_at[:, ns * 512:(ns + 1) * 512],
                                va[:, kc * 128:(kc + 1) * 128],
                                ex[:, ns * 512:(ns + 1) * 512],
                                start=(kc == 0), stop=(kc == KC - 1),
                            )
                    # normalize; denominator on row 64 (even) / row 0 (odd);
                    # even heads hop the reciprocal row to partition 0 for
                    # the gpsimd broadcast (HW broadcast reads partition 0)
                    dr = 64 if par == 0 else 0
                    if h >= QH - 2:
                        # last pair: stage PV + denominator to SBUF so the
                        # PSUM pools can close right after the final PV
                        nc.vector.tensor_copy(
                            lastU[h][r:r + 64, :], ps_at[r:r + 64, :]
                        )
                        nc.vector.tensor_copy(
                            lastD[dr:dr + 1, :], ps_at[dr:dr + 1, :]
                        )
                        if h == QH - 2:
                            # head 6's chain runs inline, from SBUF
                            normalize(h, lastD, lastU[h])
                        continue
                    normalize(h, ps_at, ps_at)

            # ================= output projection (transposed out) ==========
            # head 7's deferred normalize runs here, overlapping the early
            # contraction steps (kc2<3 don't touch attnT chunk 3)
            with tc.tile_pool(name="wop", bufs=4, space="PSUM") as wop:
                normalize(QH - 1, lastD, lastU[QH - 1])
                def op_mm(pso, mc2, ns, kc2):
                    nc.tensor.matmul(
                        pso[:, ns * 512:(ns + 1) * 512],
                        wo[:, kc2 * HID + mc2 * 128:
                           kc2 * HID + (mc2 + 1) * 128],
                        attnT[:, kc2 * S + ns * 512:
                              kc2 * S + ns * 512 + 512],
                        start=(kc2 == 0), stop=(kc2 == QC - 1),
                    )

                def op_fin(pso, mc2):
                    osb = op_.tile([128, S], F32, tag="osb")
                    nc.vector.tensor_copy(osb[:], pso[:])
                    nc.sync.dma_start(
                        out=out_d[mc2 * 128:(mc2 + 1) * 128, :], in_=osb[:]
                    )

                # first 4 output chunks pre-accumulate kc2=0..2 while head
                # 7's deferred normalize chain runs (kc2<3 don't need it)
                NPRE = 4
                pre = []
                for mc2 in range(NPRE):
                    pso = wop.tile([128, S], F32, tag="wop")
                    pre.append(pso)
                    for ns in range(2):
                        for kc2 in range(QC - 1):
                            op_mm(pso, mc2, ns, kc2)
                for mc2 in range(NPRE):
                    pso = pre[mc2]
                    for ns in range(2):
                        op_mm(pso, mc2, ns, QC - 1)
                    op_fin(pso, mc2)
                for mc2 in range(NPRE, HID // 128):
                    pso = wop.tile([128, S], F32, tag="wop")
                    for ns in range(2):
                        for kc2 in range(QC):
                            op_mm(pso, mc2, ns, kc2)
                    op_fin(pso, mc2)
    nc.finalize()
    return nc


def _rope_tables():
    inv = 1.0 / (ROPE_BASE ** (np.arange(0, HD, 2, dtype=np.float32) / HD))
    t = np.arange(S, dtype=np.float32)
    freqs = np.outer(t, inv)
    emb = np.concatenate([freqs, freqs], axis=-1)  # [S, HD]
    return np.cos(emb).astype(np.float32), np.sin(emb).astype(np.float32)


def _perm_T():
    P = np.zeros((128, 128), dtype=np.float32)
    for blk in range(2):
        o = blk * 64
        for i in range(32):
            P[o + i, o + i + 32] = -1.0
            P[o + i + 32, o + i] = 1.0
    return P.T.astype(ml_dtypes.bfloat16)


# local head order: pair mc = (kv0 head mc, kv1 head mc)
_HEAD_PERM = [0, 4, 1, 5, 2, 6, 3, 7]


def _head_cols(g):
    cols = []
    for lh in _HEAD_PERM:
        s0 = (g * QH + lh) * HD
        cols.append(np.arange(s0, s0 + HD))
    return np.concatenate(cols)


def _core_weights(g, Wq, Wk, Wv, Wo, scale):
    """Pack a core's weight slices into the SBUF tile layouts
    (partition-major, hid-chunked) so each DMA is contiguous."""
    bf = ml_dtypes.bfloat16
    cols = _head_cols(g)
    wq_c = (Wq[:, cols] * scale).astype(bf)          # [2048, 512]
    wq_pack = np.ascontiguousarray(                  # mc-major
        wq_c.reshape(HC, 128, QC, 128).transpose(1, 2, 0, 3)
        .reshape(128, QC * HC * 128))
    wk_c = Wk[:, g * KVH * HD:(g + 1) * KVH * HD].astype(bf)
    wk_pack = np.ascontiguousarray(
        wk_c.reshape(HC, 128, KVH * HD).transpose(1, 0, 2)
        .reshape(128, HC * KVH * HD))
    wv_c = Wv[:, g * KVH * HD:(g + 1) * KVH * HD].astype(bf)
    wv_pack = np.ascontiguousarray(
        wv_c.reshape(HC, 128, KVH * HD).transpose(1, 0, 2)
        .reshape(128, HC * KVH * HD))
    wo_c = Wo[cols, :].astype(bf)                    # [512, 2048]
    wo_pack = np.ascontiguousarray(
        wo_c.reshape(QC, 128, HID).transpose(1, 0, 2)
        .reshape(128, QC * HID))
    return {"wq": wq_pack, "wk": wk_pack, "wv": wv_pack, "wo": wo_pack}


def kernel(hidden_states, position_ids, attention_mask, Wq, Wk, Wv, Wo,
           _trace=False):
    global LAST_RESULT
    bf = ml_dtypes.bfloat16
    hidden_states = np.asarray(hidden_states, dtype=np.float32)
    Wq = np.asarray(Wq, dtype=np.float32)
    Wk = np.asarray(Wk, dtype=np.float32)
    Wv = np.asarray(Wv, dtype=np.float32)
    Wo = np.asarray(Wo, dtype=np.float32)
    mask = np.asarray(attention_mask, dtype=np.float32)
    pos = np.asarray(position_ids).astype(np.int64)

    use_mask = bool(np.any(mask))
    key = use_mask
    if key not in _CACHE:
        _CACHE[key] = _build(use_mask)
    nc = _CACHE[key]

    cos_t, sin_t = _rope_tables()
    permT = _perm_T()
    scale = 1.0 / np.sqrt(HD)

    hsT_b = [np.ascontiguousarray(hidden_states[b].T).astype(bf)
             for b in range(B)]
    cos2_b, sin2_b = [], []
    for b in range(B):
        cos2_b.append(np.ascontiguousarray(
            np.tile(cos_t[pos[b]].T, (2, 1))).astype(np.float32))
        sin2_b.append(np.ascontiguousarray(
            np.tile(sin_t[pos[b]].T, (2, 1))).astype(np.float32))
    if use_mask:
        maskT_full = np.ascontiguousarray(
            np.maximum(mask[:, 0], NEG_BIG).transpose(0, 2, 1)).astype(bf)

    in_maps = []
    for c in range(8):
        b, g = c // G, c % G
        m = _core_weights(g, Wq, Wk, Wv, Wo, scale)
        m.update({
            "hsT": hsT_b[b],
            "permT": permT,
            "cos2": cos2_b[b],
            "sin2": sin2_b[b],
        })
        if use_mask:
            m["maskT"] = maskT_full[b]
        in_maps.append(m)

    res = run_bass_kernel_spmd(nc, in_maps, core_ids=list(range(8)),
                               trace=_trace)
    LAST_RESULT = res
    out = np.zeros((B, S, HID), dtype=np.float32)
    for c in range(8):
        out[c // G] += res.results[c]["out"].T
    return out
